# revision 31
# baseline (speedup 1.0000x reference)
"""Trainium2 Bass kernel for causal multi-head attention (dense transformer block).

Problem (hardcoded): x [2, 2048, 1024], 16 heads x 64 dh, causal attention,
fp32 I/O. Sharding: 8 cores = 2 batches x 4 head-groups. Each core computes 4
heads for one batch plus a partial output projection [2048, 1024]; the host
sums the 4 partials per batch and adds b_O.

Everything on-device is computed in "transposed" orientation so no transposes
are needed anywhere:
  x^T (host-pretransposed)  ->  Q^T, K^T [dh, s] and V [s, dh] via matmuls
  S^T[k, q] = K Q^T         ->  P^T = exp(S^T / 8) (causal-masked pre-exp)
  Z^T[dh, q] = V^T P^T      ->  normalized by column sums (ones-matmul)
  O[s, :]   = (Z^T)^T W_O   (Z^T is directly the lhsT of the O-projection)

Heads are processed in pairs: QK^T packs 2 heads in row-groups (0-63 / 64-127)
of the PE array, PV packs 2 heads in column-groups -- both run concurrently.

Schedule: input DMAs are host-prearranged to be contiguous and issued
critical-path-first across two queues; a warm-up matmul burst flips the PE
HAM clock gate to 2.4 GHz during the input load; pair-1 projections are
emitted inside pair-0's attention and the output projection per q-block
inside pair-1's attention, so the PE stays saturated while the scalar
engine grinds softmax exps and the output DMA drains behind compute.
"""

import os
from contextlib import ExitStack

import numpy as np

import concourse.tile as tile
from concourse import bacc, mybir
from concourse.bass_utils import run_bass_kernel_spmd

# problem constants
B, S, DM, H, DH = 2, 2048, 1024, 16, 64
P = 128          # partitions
QB = 512         # q block (matmul moving free dim)
NKT = S // P     # 16 k tiles
NQB = S // QB    # 4 q blocks
NDM = DM // P    # 8 d_model tiles
HPC = 4          # heads per core
NCORES = 8

F32 = mybir.dt.float32
BF16 = mybir.dt.bfloat16

_PROGRAM_CACHE = {}
LAST_RESULTS = None  # BassKernelResults of the most recent run (for test.py)


def _mm(nc, out, lhsT, rhs, start, stop, skip=False):
    # skip_group_check: the sim's psum-group tracker doesn't distinguish
    # partition ranges; our concurrent groups in one bank are partition-disjoint
    # (rows 0-63 vs 64-127), which the per-partition zeroing model handles.
    return nc.tensor.matmul(
        out, lhsT, rhs, start=start, stop=stop, skip_group_check=skip
    )


def _chain(insts):
    """Ordering-only PE edges so matmuls alternating between row/column
    groups stay adjacent and run concurrently on the array."""
    from concourse.tile import add_dep_helper

    for a, b in zip(insts[1:], insts):
        add_dep_helper(a.ins, b.ins, sync=False, reason="pack-pair order")


def build_program():
    """Build the single-core SPMD Bass program (same program on all 8 cores)."""
    if "k" in _PROGRAM_CACHE:
        return _PROGRAM_CACHE["k"]

    nc = bacc.Bacc(
        "TRN2", target_bir_lowering=False, debug=False, num_devices=NCORES
    )

    # ---- DRAM I/O (per-core shards, prearranged on host for contiguous DMA) ----
    xt_d = nc.dram_tensor("xt", [P, NQB, NDM, QB], BF16, kind="ExternalInput")
    wq_d = nc.dram_tensor("wq", [P, NDM, HPC * DH], BF16, kind="ExternalInput")
    wk_d = nc.dram_tensor("wk", [P, NDM, HPC * DH], BF16, kind="ExternalInput")
    wv_d = nc.dram_tensor("wv", [P, NDM, HPC * DH], BF16, kind="ExternalInput")
    wo_d = nc.dram_tensor("wo", [P, 2, DM], BF16, kind="ExternalInput")
    bq_d = nc.dram_tensor("bq", [2, P], F32, kind="ExternalInput")
    bk_d = nc.dram_tensor("bk", [2, P], F32, kind="ExternalInput")
    bv_d = nc.dram_tensor("bv", [P, HPC * DH], F32, kind="ExternalInput")
    tri_d = nc.dram_tensor("tri", [P, P], BF16, kind="ExternalInput")
    out_d = nc.dram_tensor("out", [S, DM], F32, kind="ExternalOutput")

    with tile.TileContext(nc) as tc, ExitStack() as ctx:
        const = ctx.enter_context(tc.tile_pool(name="const", bufs=1))
        persist = ctx.enter_context(tc.tile_pool(name="persist", bufs=1))

        # ---- constants / small tiles ----
        ones64 = const.tile([P, 64], BF16, name="ones64", tag="ones64")
        nc.gpsimd.memset(ones64[:], 1.0)
        warm_rhs = const.tile([P, QB], BF16, name="warm_rhs", tag="warm")
        nc.vector.memset(warm_rhs[:], 0.0)
        # lower-triangle 0/1 mask for the 128x128 diagonal block of a k-tile
        tri_sb = const.tile([P, P], BF16, name="tri_sb", tag="tri")
        bq_sb = const.tile([P, 2], F32, name="bq_sb", tag="bq")
        bk_sb = const.tile([P, 2], F32, name="bk_sb", tag="bk")
        bv_sb = const.tile([P, HPC * DH], F32, name="bv_sb", tag="bv")

        # ---- persistent activations ----
        xt_sb = persist.tile([P, NQB, NDM, QB], BF16, name="xt_sb", tag="xt")
        w_sb = {
            w: persist.tile([P, NDM, HPC * DH], BF16, name=f"{w}_sb", tag=w)
            for w in ("wq", "wk", "wv")
        }
        wo_sb = persist.tile([P, 2, DM], BF16, name="wo_sb", tag="wo")
        qt_sb = [
            persist.tile([P, S], BF16, name=f"qt{p}", tag=f"qt{p}") for p in range(2)
        ]
        kt_sb = [
            persist.tile([P, S], BF16, name=f"kt{p}", tag=f"kt{p}") for p in range(2)
        ]
        v_sb = [
            persist.tile([P, NKT, P], BF16, name=f"v{p}", tag=f"v{p}")
            for p in range(2)
        ]
        zt_sb = [
            persist.tile([P, S], BF16, name=f"zt{p}", tag=f"zt{p}") for p in range(2)
        ]

        # ---- input DMAs: x chunks on the sync queue, weights concurrently
        # on the scalar hwdge queue, so the critical path (wq + x ch0) is
        # two parallel ~1MB transfers instead of a serial chain ----
        nc.sync.dma_start(out=xt_sb[:, 0], in_=xt_d[:, 0])
        nc.scalar.dma_start(out=w_sb["wq"][:], in_=wq_d[:, :])
        nc.scalar.dma_start(out=w_sb["wk"][:], in_=wk_d[:, :])
        for ch in range(1, NQB):
            nc.sync.dma_start(out=xt_sb[:, ch], in_=xt_d[:, ch])
        nc.scalar.dma_start(out=w_sb["wv"][:], in_=wv_d[:, :])
        nc.sync.dma_start(out=wo_sb[:], in_=wo_d[:, :])
        nc.scalar.dma_start(out=tri_sb[:], in_=tri_d[:, :])
        for p in range(2):
            nc.scalar.dma_start(out=bq_sb[:, p : p + 1], in_=bq_d[p : p + 1, :])
            nc.scalar.dma_start(out=bk_sb[:, p : p + 1], in_=bk_d[p : p + 1, :])
        nc.scalar.dma_start(out=bv_sb[:], in_=bv_d[:, :])

        # ======= psum pools =======
        # sp: score tiles only (2-bank tiles); fp: 1-bank tiles for the
        # projection / output-projection filler streams -- a separate pool so
        # filler matmuls never wait on score-tile consumers (exp)
        sp = ctx.enter_context(tc.tile_pool(name="sp", bufs=2, space="PSUM"))
        fp = ctx.enter_context(tc.tile_pool(name="fp", bufs=2, space="PSUM"))
        zp = ctx.enter_context(tc.tile_pool(name="zp", bufs=1, space="PSUM"))
        dp = ctx.enter_context(tc.tile_pool(name="dp", bufs=1, space="PSUM"))
        ppool = ctx.enter_context(tc.tile_pool(name="ppool", bufs=8))
        bcpool = ctx.enter_context(tc.tile_pool(name="bcpool", bufs=2))
        ost = ctx.enter_context(tc.tile_pool(name="ost", bufs=4))

        # ---- warm-up burst: no data deps on DMAs, flips the PE HAM gate to
        # 2.4 GHz while inputs load ----
        # ~16 x 427ns cold matmuls bridge the input-DMA wait with sustained PE
        # activity, so the HAM clock gate is already at 8/8 when the first
        # projection matmul issues
        warm_mms = []
        for _ in range(9):
            for _b in range(2):
                wt = fp.tile([P, QB], F32, name="warm", tag="f")
                warm_mms.append(
                    _mm(nc, wt[:], warm_rhs[:, 0:P], warm_rhs[:],
                        start=True, stop=True)
                )
        _chain(warm_mms)
        # preload the exp table during the load stall
        pre = ppool.tile([P, 2, QB], BF16, name="pre", tag="pt")
        nc.scalar.activation(
            pre[:, 0, 0:64], warm_rhs[:, 0:64],
            mybir.ActivationFunctionType.Exp, scale=0.125,
        )

        def qk_item(p, ch, wname):
            # one Q^T or K^T chunk for pair p: [dh-pair (128), QB q-cols];
            # rows 0-63 = head 2p, 64-127 = head 2p+1
            dst, bias = ((qt_sb, bq_sb) if wname == "wq" else (kt_sb, bk_sb))
            qp = fp.tile([P, QB], F32, name="qp", tag="f")
            for t in range(NDM):
                _mm(
                    nc,
                    qp[:],
                    w_sb[wname][:, t, p * P : (p + 1) * P],
                    xt_sb[:, ch, t, :],
                    start=(t == 0),
                    stop=(t == NDM - 1),
                )
            nc.vector.tensor_scalar_add(
                dst[p][:, ch * QB : (ch + 1) * QB],
                qp[:],
                bias[:, p : p + 1],
            )

        def v_item(st):
            # V: [seq, head-pair dh] for one 128-row tile, both pairs
            vp = fp.tile([P, QB], F32, name="vp", tag="f")
            for t in range(NDM):
                _mm(
                    nc,
                    vp[:, 0 : HPC * DH],
                    xt_sb[:, st // 4, t, (st % 4) * P : (st % 4 + 1) * P],
                    w_sb["wv"][:, t, :],
                    start=(t == 0),
                    stop=(t == NDM - 1),
                )
            for p in range(2):
                nc.vector.tensor_add(
                    v_sb[p][:, st, :],
                    vp[:, p * P : (p + 1) * P],
                    bv_sb[:, p * P : (p + 1) * P],
                )

        def oproj_item(st, dma_eng=None):
            # output-projection partial O[st*128:(st+1)*128, :], staged to
            # fp32 SBUF and DMA'd out
            ot = ost.tile([P, 2, QB], F32, name="ot", tag="ot")
            for nn in range(2):
                ops = fp.tile([P, QB], F32, name="ops", tag="f")
                for pp in range(2):
                    _mm(
                        nc,
                        ops[:],
                        zt_sb[pp][:, st * P : (st + 1) * P],
                        wo_sb[:, pp, nn * QB : (nn + 1) * QB],
                        start=(pp == 0),
                        stop=(pp == 1),
                    )
                nc.vector.tensor_copy(ot[:, nn, :], ops[:])
            (dma_eng or nc.sync).dma_start(
                out=out_d[st * P : (st + 1) * P, :],
                in_=ot[:],
            )

        def attention(p, qb, fillers=(), tail=None):
            q0 = qb * QB
            nk = (qb + 1) * (QB // P)  # k tiles in causal range
            zps = zp.tile([P, QB], F32, name="zps", tag="z")
            dnb = dp.tile([P, QB], F32, name="dnb", tag="d")

            def pv_dnb(pA, pB, kg):
                # PV (column-packed heads) + softmax denominators: the
                # ones-matmul sums P over k AND broadcasts over the 64
                # rows of each head half, accumulated in PSUM; all read
                # only the valid q range of their k-tile
                for j in range(2):
                    kt = kg * 2 + j
                    c0 = max(kt * P - q0, 0)
                    # (zpsA|zpsB) and (dnbA|dnbB) are column-group-disjoint
                    # pairs -- each pair runs concurrently on the PE array
                    _chain([
                        _mm(
                            nc, zps[0:64, c0:QB], v_sb[p][:, kt, 0:64],
                            pA[:, j, c0:QB],
                            start=(kt == 0), stop=(kt == nk - 1), skip=True,
                        ),
                        _mm(
                            nc, zps[64:P, c0:QB], v_sb[p][:, kt, 64:P],
                            pB[:, j, c0:QB],
                            start=(kt == 0), stop=(kt == nk - 1), skip=True,
                        ),
                        _mm(
                            nc, dnb[0:64, c0:QB], ones64[:], pA[:, j, c0:QB],
                            start=(kt == 0), stop=(kt == nk - 1), skip=True,
                        ),
                        _mm(
                            nc, dnb[64:P, c0:QB], ones64[:], pB[:, j, c0:QB],
                            start=(kt == 0), stop=(kt == nk - 1), skip=True,
                        ),
                    ])

            # pace the independent filler items evenly across the kg loop,
            # emitted between scores and PV so the PE queue has ready work
            # while the scalar engine runs this kg's exps
            nkg = nk // 2
            fillers = list(fillers)
            nf = len(fillers)

            for kg in range(nkg):
                # offs[j]: first valid q column of k-tile kg*2+j
                offs = [kg * 2 * P + j * P - q0 for j in range(2)]
                band = offs[0] >= 0
                deep = band and offs[0] >= 2 * P  # o=1 band k-group
                # e0: first column of the exp (and scores-write) range.
                # Scores are computed from e0 for BOTH j so the fused exp
                # never reads psum this allocation didn't write.
                e0 = max(offs[0], 0)
                sA = sp.tile([P, 2, QB], F32, name="sA", tag="s")
                sB = sp.tile([P, 2, QB], F32, name="sB", tag="s")
                for j in range(2):
                    _chain([
                        _mm(
                            nc,
                            stile[:, j, e0:QB],
                            kt_sb[p][rows, (kg * 2 + j) * P : (kg * 2 + j + 1) * P],
                            qt_sb[p][rows, q0 + e0 : q0 + QB],
                            start=True,
                            stop=True,
                        )
                        for rows, stile in ((slice(0, 64), sA), (slice(64, P), sB))
                    ])
                for f in fillers[nf * kg // nkg : nf * (kg + 1) // nkg]:
                    f()
                pA = ppool.tile([P, 2, QB], BF16, name="pA", tag="pt")
                pB = ppool.tile([P, 2, QB], BF16, name="pB", tag="pt")
                # exp(S/sqrt(dh)); scale folded into ACT.  One fused
                # instruction per head covering [e0:QB] of both k-tiles.
                nc.scalar.activation(
                    pA[:, :, e0:QB], sA[:, :, e0:QB],
                    mybir.ActivationFunctionType.Exp, scale=0.125,
                )
                nc.scalar.activation(
                    pB[:, :, e0:QB], sB[:, :, e0:QB],
                    mybir.ActivationFunctionType.Exp, scale=0.125,
                )
                if band:
                    # causal mask: only the 128x128 diagonal block of each
                    # k-tile is partially masked (columns left of it aren't
                    # read by PV, columns right of it are fully valid), so
                    # multiply just that block by the triangle mask
                    for px in (pA, pB):
                        for j in range(2):
                            nc.vector.tensor_mul(
                                px[:, j, offs[j] : offs[j] + P],
                                px[:, j, offs[j] : offs[j] + P],
                                tri_sb[:],
                            )
                pv_dnb(pA, pB, kg)

            bcs = bcpool.tile([P, QB], F32, name="bcs", tag="bcs")
            bcr = bcpool.tile([P, QB], F32, name="bcr", tag="bcr")
            nc.vector.reciprocal_approx_accurate(
                out=bcr[:], in_=dnb[:], scratch=bcs[:]
            )
            if tail is None:
                nc.vector.tensor_mul(zt_sb[p][:, q0 : q0 + QB], zps[:], bcr[:])
            else:
                # split the normalize per 128-row tile and chase each piece
                # with its output projection, so the kernel tail pipelines
                for st in range(qb * 4, (qb + 1) * 4):
                    r = (st % 4) * P
                    nc.vector.tensor_mul(
                        zt_sb[p][:, q0 + r : q0 + r + P],
                        zps[:, r : r + P],
                        bcr[:, r : r + P],
                    )
                    tail(st)

        # ---- interleaved emission: keep the PE saturated while the scalar
        # engine grinds exps; output DMA drains behind pair-1 attention ----
        def fi(fn, *a):
            return lambda: fn(*a)

        # attention(0,0) starts as soon as chunk-0 projections + v0-3 exist;
        # every remaining projection / output-projection is a paced filler
        # inside a later attention block, placed just before its first reader
        qk_item(0, 0, "wq")
        qk_item(0, 0, "wk")
        for st in range(4):
            v_item(st)
        attention(0, 0, [fi(qk_item, 0, 1, "wq"), fi(qk_item, 0, 1, "wk")])
        attention(0, 1, [fi(v_item, 4), fi(v_item, 5), fi(qk_item, 0, 2, "wq"),
                         fi(v_item, 6), fi(v_item, 7), fi(qk_item, 0, 2, "wk")])
        attention(0, 2, [fi(v_item, 8), fi(v_item, 9), fi(qk_item, 0, 3, "wq"),
                         fi(v_item, 10), fi(v_item, 11), fi(qk_item, 0, 3, "wk")])
        attention(0, 3, [fi(v_item, 12), fi(v_item, 13), fi(v_item, 14),
                         fi(v_item, 15), fi(qk_item, 1, 0, "wq"),
                         fi(qk_item, 1, 0, "wk"), fi(qk_item, 1, 1, "wq"),
                         fi(qk_item, 1, 1, "wk")])
        attention(1, 0, [fi(qk_item, 1, 2, "wq"), fi(qk_item, 1, 2, "wk")])
        attention(1, 1, [fi(qk_item, 1, 3, "wq"), fi(qk_item, 1, 3, "wk"),
                         fi(oproj_item, 0), fi(oproj_item, 1)])
        attention(1, 2, [fi(oproj_item, st) for st in range(2, 6)])
        # tail DMAs alternate between the sync and scalar queues so the last
        # four 512KB output transfers drain two-at-a-time
        attention(1, 3, [fi(oproj_item, st) for st in range(6, 12)],
                  tail=lambda st: oproj_item(
                      st, nc.scalar if st % 2 else nc.sync))

    nc.compile()
    _PROGRAM_CACHE["k"] = nc
    return nc


def make_in_maps(normalized_resid_pre, W_Q, W_K, W_V, W_O, b_Q, b_K, b_V, b_O):
    """Shard + prearrange the full inputs into per-core input maps."""
    import ml_dtypes  # noqa: F401  (registers bfloat16 with numpy)

    np_bf16 = np.dtype("bfloat16")

    x = np.asarray(normalized_resid_pre, dtype=np.float32)
    W_Q = np.asarray(W_Q, dtype=np.float32)
    W_K = np.asarray(W_K, dtype=np.float32)
    W_V = np.asarray(W_V, dtype=np.float32)
    W_O = np.asarray(W_O, dtype=np.float32)
    b_Q = np.asarray(b_Q, dtype=np.float32)
    b_K = np.asarray(b_K, dtype=np.float32)
    b_V = np.asarray(b_V, dtype=np.float32)

    # xt[p, ch, t, q] = x[b][ch*QB + q, t*P + p]
    xt = [
        np.ascontiguousarray(
            x[b].T.reshape(NDM, P, NQB, QB).transpose(1, 2, 0, 3)
        ).astype(np_bf16)
        for b in range(B)
    ]
    # lower-triangle (q >= k) 0/1 mask for a 128x128 diagonal block
    kp = np.arange(P)[:, None]
    qc = np.arange(P)[None, :]
    tri = (qc >= kp).astype(np.float32).astype(np_bf16)  # [P, P]

    def warr(w):
        # [DM, HPC*DH] -> [P, NDM, HPC*DH]
        return np.ascontiguousarray(
            w.reshape(NDM, P, HPC * DH).transpose(1, 0, 2)
        ).astype(np_bf16)

    in_maps = []
    for c in range(NCORES):
        b = c // (NCORES // B)
        heads = [HPC * (c % (NCORES // B)) + i for i in range(HPC)]
        wq = warr(np.concatenate([W_Q[h] for h in heads], axis=1))
        wk = warr(np.concatenate([W_K[h] for h in heads], axis=1))
        wv = warr(np.concatenate([W_V[h] for h in heads], axis=1))
        wo_cat = np.concatenate([W_O[h] for h in heads], axis=0)  # [2P, DM]
        wo = np.ascontiguousarray(
            wo_cat.reshape(2, P, DM).transpose(1, 0, 2)
        ).astype(np_bf16)
        bq = np.stack(
            [
                np.concatenate([b_Q[heads[0]], b_Q[heads[1]]]),
                np.concatenate([b_Q[heads[2]], b_Q[heads[3]]]),
            ]
        ).astype(np.float32)
        bk = np.stack(
            [
                np.concatenate([b_K[heads[0]], b_K[heads[1]]]),
                np.concatenate([b_K[heads[2]], b_K[heads[3]]]),
            ]
        ).astype(np.float32)
        bv = np.tile(
            np.concatenate([b_V[h] for h in heads])[None, :], (P, 1)
        ).astype(np.float32)
        in_maps.append(
            {
                "xt": xt[b],
                "wq": wq, "wk": wk, "wv": wv, "wo": wo,
                "bq": bq, "bk": bk, "bv": bv,
                "tri": tri,
            }
        )
    return in_maps


def kernel(normalized_resid_pre, W_Q, W_K, W_V, W_O, b_Q, b_K, b_V, b_O):
    global LAST_RESULTS
    nc = build_program()
    in_maps = make_in_maps(
        normalized_resid_pre, W_Q, W_K, W_V, W_O, b_Q, b_K, b_V, b_O
    )
    trace = os.environ.get("ATTN_TRACE", "0") == "1"
    res = run_bass_kernel_spmd(nc, in_maps, list(range(NCORES)), trace=trace)
    LAST_RESULTS = res

    b_O = np.asarray(b_O, dtype=np.float32)
    parts = [np.asarray(res.results[c]["out"], dtype=np.float64) for c in range(NCORES)]
    npc = NCORES // B  # cores per batch
    out = np.stack(
        [sum(parts[b * npc : (b + 1) * npc]) + b_O for b in range(B)]
    )
    return out.astype(np.float32)
